# revision 18
# baseline (speedup 1.0000x reference)
"""Bahdanau additive attention on 8 Trainium2 NeuronCores.

Problem shapes (hardcoded): B=4, T=128, S=512, H=256, fp32.

Sharding: data-parallel over (batch, T-half): core c handles b = c//2,
t in [64*(c%2), 64*(c%2)+64).  Every core runs the same SPMD program on
its own shard; weights are replicated.  No collectives.

Per-core algorithm (T_loc=64, S=512, H=256):
  peT[h,s] = (Wh @ enc^T)[h,s]        fp32 matmuls
  pqT[h,t] = (Ws @ q^T)[h,t]          fp32 matmuls
  For each t:  Y[h,s] = peT[h,s] + pqT[h,t]   (tensor_scalar_add with the
               pq column as the per-partition scalar operand, alternating
               between the DVE and Pool engines)
  X = tanh(Y)                          ACT on big (128 x 8192) tiles, fp16 out
  e[t,s] = sum_h v[h]*X[h,s]           PE fp16, shifted-vz stationary trick
  P = exp(e)                           no max-subtraction: |e| <= ||v||_1
  PT = P^T (PE transpose), PTm = PT * mask[s]  (per-partition mul)
  Z[t] = sum_s PTm[s,t]*mask[s]        PE matmul, out (64x1) t-on-partition
  c[t,:] = (1/Z[t]) * sum_s PTm[s,t]*enc[s,:]
  attn = tanh([q,c] @ Wout^T)          catT = [qT; cT], fp32 matmuls
"""

import numpy as np

B, T, S, H = 4, 128, 512, 256
TLOC = 64          # T rows per core
NCORES = 8
TGS = 16           # t's per tanh group
NG = TLOC // TGS   # 8 groups
P = 128            # partitions
HC = H // P        # 2 h-chunks
SB = S // P        # 4 s-blocks
FC = (2 * H) // P  # 4 f-chunks of cat=[q,c]

_CACHE = {}


def build_module():
    """Build + compile the SPMD Bass module (same program for all cores)."""
    if "nc" in _CACHE:
        return _CACHE["nc"]

    import concourse.bass as bass
    import concourse.tile as tile
    from concourse import bacc, mybir

    f32 = mybir.dt.float32
    f16 = mybir.dt.float16
    f32r = mybir.dt.float32r
    AF = mybir.ActivationFunctionType

    nc = bacc.Bacc(
        "TRN2",
        target_bir_lowering=False,
        debug=False,
        enable_asserts=False,
        num_devices=NCORES,
    )

    d_qT = nc.dram_tensor("qT_l", (H, TLOC), f32, kind="ExternalInput").ap()
    d_encT = nc.dram_tensor("encT_l", (H, S), f32, kind="ExternalInput").ap()
    d_enc = nc.dram_tensor("enc_l", (S, H), f32, kind="ExternalInput").ap()
    d_wsT = nc.dram_tensor("wsT", (H, H), f32, kind="ExternalInput").ap()
    d_whT = nc.dram_tensor("whT", (H, H), f32, kind="ExternalInput").ap()
    d_woutT = nc.dram_tensor("woutT", (2 * H, H), f32, kind="ExternalInput").ap()
    d_v16 = nc.dram_tensor("v16", (HC * P, 16), f16, kind="ExternalInput").ap()
    d_mask = nc.dram_tensor("maskc", (P, SB), f32, kind="ExternalInput").ap()
    d_ident = nc.dram_tensor("ident", (TLOC, TLOC), f32, kind="ExternalInput").ap()
    d_out = nc.dram_tensor("out_l", (TLOC, H), f32, kind="ExternalOutput").ap()

    with tile.TileContext(nc) as tc:
        from contextlib import ExitStack

        with ExitStack() as ctx:
            consts = ctx.enter_context(tc.tile_pool(name="consts", bufs=1))
            proj = ctx.enter_context(tc.tile_pool(name="proj", bufs=1))
            ypool = ctx.enter_context(tc.tile_pool(name="ypool", bufs=1))
            xpool = ctx.enter_context(tc.tile_pool(name="xpool", bufs=2))
            tail = ctx.enter_context(tc.tile_pool(name="tail", bufs=1))
            psA = ctx.enter_context(tc.tile_pool(name="psA", bufs=1, space="PSUM"))
            psE = ctx.enter_context(tc.tile_pool(name="psE", bufs=1, space="PSUM"))
            psE8 = ctx.enter_context(tc.tile_pool(name="psE8", bufs=3, space="PSUM"))
            psT = ctx.enter_context(tc.tile_pool(name="psT", bufs=3, space="PSUM"))

            # ---- load constants / inputs ----
            ws_sb = []
            wh_sb = []
            wout_sb = []
            qT_sb = []
            encT_sb = []
            enc_sb = []
            for kc in range(HC):
                t1 = consts.tile([P, H], f32, name=f"ws_sb{kc}")
                nc.sync.dma_start(t1[:], d_wsT[kc * P:(kc + 1) * P, :])
                ws_sb.append(t1)
                t2 = consts.tile([P, H], f32, name=f"wh_sb{kc}")
                nc.sync.dma_start(t2[:], d_whT[kc * P:(kc + 1) * P, :])
                wh_sb.append(t2)
                t3 = consts.tile([P, TLOC], f32, name=f"qT_sb{kc}")
                nc.sync.dma_start(t3[:], d_qT[kc * P:(kc + 1) * P, :])
                qT_sb.append(t3)
                t4 = consts.tile([P, S], f32, name=f"encT_sb{kc}")
                nc.sync.dma_start(t4[:], d_encT[kc * P:(kc + 1) * P, :])
                encT_sb.append(t4)
            for fc in range(FC):
                t5 = consts.tile([P, H], f32, name=f"wout_sb{fc}")
                nc.sync.dma_start(t5[:], d_woutT[fc * P:(fc + 1) * P, :])
                wout_sb.append(t5)
            for sb in range(SB):
                t6 = consts.tile([P, H], f32, name=f"enc_sb{sb}")
                nc.sync.dma_start(t6[:], d_enc[sb * P:(sb + 1) * P, :])
                enc_sb.append(t6)
            v16_sb = []
            for hc in range(HC):
                t6v = consts.tile([P, 16], f16, name=f"v16_sb{hc}")
                nc.sync.dma_start(t6v[:], d_v16[hc * P:(hc + 1) * P, :])
                v16_sb.append(t6v)
            mask_sb = consts.tile([P, SB], f32)
            nc.sync.dma_start(mask_sb[:], d_mask[:, :])
            ident_sb = consts.tile([TLOC, TLOC], f32)
            nc.sync.dma_start(ident_sb[:], d_ident[:, :])

            # ---- projections ----
            # peT[oc] (128 x 512): peT[o,s] = sum_h Wh[o,h] * encT[h,s]
            peT_sb = []
            for oc in range(HC):
                pe_ps = psA.tile([P, S], f32, name=f"pe_ps{oc}", tag="pe_ps")
                for kc in range(HC):
                    nc.tensor.matmul(
                        pe_ps[:],
                        lhsT=wh_sb[kc][:, oc * P:(oc + 1) * P],
                        rhs=encT_sb[kc][:],
                        start=(kc == 0),
                        stop=(kc == HC - 1),
                    )
                t7 = proj.tile([P, S], f32, name=f"peT_sb{oc}")
                nc.scalar.copy(t7[:], pe_ps[:])
                peT_sb.append(t7)

            # pqT[oc] (128 x 64): pqT[o,t] = sum_h Ws[o,h] * qT[h,t]  (fp32)
            pqT_sb = []
            for oc in range(HC):
                pq_ps = psT.tile([P, TLOC], f32, name=f"pq_ps{oc}", tag="tail")
                for kc in range(HC):
                    nc.tensor.matmul(
                        pq_ps[:],
                        lhsT=ws_sb[kc][:, oc * P:(oc + 1) * P],
                        rhs=qT_sb[kc][:],
                        start=(kc == 0),
                        stop=(kc == HC - 1),
                    )
                t8 = proj.tile([P, TLOC], f32, name=f"pqT_sb{oc}")
                nc.scalar.copy(t8[:], pq_ps[:])
                pqT_sb.append(t8)

            # ---- main loop: Y = pe + pq_t ; X = tanh(Y) ; e = v^T X ----
            # e rows come from M=8 matmuls with a shifted-column stationary
            # operand: v16[hc] is (128 x 16) with v[hc] at column 8, so
            # lhsT = v16[:, 8-jj:16-jj] has v in column jj -> the matmul
            # deposits row jj = v^T X_t (zeros elsewhere) of an (8 x 512)
            # PSUM tile, accumulating over hc.  8-row tiles are dense in
            # partitions, so a single DVE copy moves each to SBUF and PE
            # mini-transposes assemble eT (s-major) for the softmax tail.
            GS = [8, 8, 16, 16, 16]   # staggered group sizes (sum 64)
            eT_ps = psE.tile([P, SB * TLOC], f32)  # (128 x 256) eT cols
            e8_sbs = []
            t0g = 0
            for g, tgs in enumerate(GS):
                xs = []
                for hc in range(HC):
                    y = ypool.tile([P, TGS * S], f32, name=f"y_{g}_{hc}",
                                   tag=f"y{hc}")
                    for j in range(tgs):
                        t = t0g + j
                        nc.vector.tensor_scalar_add(
                            y[:, j * S:(j + 1) * S],
                            peT_sb[hc][:],
                            pqT_sb[hc][:, t:t + 1],
                        )
                    x = xpool.tile([P, TGS * S], f16, name=f"x_{g}_{hc}",
                                   tag=f"x{hc}")
                    nc.scalar.activation(x[:, 0:tgs * S], y[:, 0:tgs * S],
                                         AF.Tanh)
                    xs.append(x)
                for u0 in range(tgs // 8):
                    u = (t0g // 8) + u0
                    et = psE8.tile([8, S], f32, name=f"e_{u}", tag="e_rows")
                    for hc in range(HC):
                        for jj in range(8):
                            j = u0 * 8 + jj
                            nc.tensor.matmul(
                                et[:, :],
                                lhsT=v16_sb[hc][:, 8 - jj:16 - jj],
                                rhs=xs[hc][:, j * S:(j + 1) * S],
                                start=(hc == 0 and jj == 0),
                                stop=(hc == HC - 1 and jj == 7),
                                skip_group_check=True,
                            )
                    e8 = tail.tile([8, S], f32, name=f"e8_{u}", tag="e8sb",
                                   bufs=3)
                    nc.vector.tensor_copy(e8[:], et[:])
                    for sb in range(SB):
                        nc.tensor.transpose(
                            eT_ps[:, sb * TLOC + u * 8:sb * TLOC + u * 8 + 8],
                            e8[:, sb * P:(sb + 1) * P],
                            ident_sb[0:8, 0:8],
                        )
                t0g += tgs

            # ---- softmax tail ----
            # eT_ps is (s-part x t-free); exp all chunks in one ACT op,
            # then per-partition masking per s-block chunk.
            pt_sb = tail.tile([P, SB * TLOC], f32)
            nc.scalar.activation(pt_sb[:], eT_ps[:], AF.Exp)
            ptm_sb = []
            for sb in range(SB):
                t9 = tail.tile([P, TLOC], f32, name=f"ptm_sb{sb}")
                nc.vector.tensor_scalar_mul(
                    t9[:],
                    pt_sb[:, sb * TLOC:(sb + 1) * TLOC],
                    mask_sb[:, sb:sb + 1],
                )
                ptm_sb.append(t9)

            z_ps = psT.tile([TLOC, 1], f32, tag="tail")
            for sb in range(SB):
                nc.tensor.matmul(
                    z_ps[:],
                    lhsT=ptm_sb[sb][:],
                    rhs=mask_sb[:, sb:sb + 1],
                    start=(sb == 0),
                    stop=(sb == SB - 1),
                )
            r_sb = tail.tile([TLOC, 1], f32)
            nc.vector.reciprocal(r_sb[:], z_ps[:])

            cun_ps = psT.tile([TLOC, H], f32, tag="tail")
            for sb in range(SB):
                nc.tensor.matmul(
                    cun_ps[:],
                    lhsT=ptm_sb[sb][:],
                    rhs=enc_sb[sb][:],
                    start=(sb == 0),
                    stop=(sb == SB - 1),
                )
            c_sb = tail.tile([TLOC, H], f32)
            nc.vector.tensor_scalar_mul(c_sb[:], cun_ps[:], r_sb[:])

            ct_ps = psT.tile([P, 2 * TLOC], f32, tag="tail")
            for i in range(HC):
                nc.tensor.transpose(
                    ct_ps[:, i * TLOC:(i + 1) * TLOC],
                    c_sb[:, i * P:(i + 1) * P],
                    ident_sb[:],
                )
            ct_sb = tail.tile([P, 2 * TLOC], f32)
            nc.vector.tensor_copy(ct_sb[:], ct_ps[:])

            attn_ps = psT.tile([TLOC, H], f32, tag="tail")
            cat_tiles = [
                qT_sb[0][:],
                qT_sb[1][:],
                ct_sb[:, 0:TLOC],
                ct_sb[:, TLOC:2 * TLOC],
            ]
            for fc in range(FC):
                nc.tensor.matmul(
                    attn_ps[:],
                    lhsT=cat_tiles[fc],
                    rhs=wout_sb[fc][:],
                    start=(fc == 0),
                    stop=(fc == FC - 1),
                )
            o_sb = tail.tile([TLOC, H], f32)
            nc.scalar.activation(o_sb[:], attn_ps[:], AF.Tanh)
            nc.sync.dma_start(d_out[:, :], o_sb[:])

    nc.compile()
    _CACHE["nc"] = nc
    return nc


def make_in_maps(query, encoder_outputs, src_lengths, Ws, Wh, v, Wout):
    """Host-side shard/layout prep: per-core input dict (all fp32, C-order)."""
    f = np.float32
    wsT = np.ascontiguousarray(np.asarray(Ws, f).T)
    whT = np.ascontiguousarray(np.asarray(Wh, f).T)
    woutT = np.ascontiguousarray(np.asarray(Wout, f).T)
    v16 = np.zeros((HC * P, 16), np.float16)
    for hc in range(HC):
        v16[hc * P:(hc + 1) * P, 8] = np.asarray(v, np.float32)[
            hc * P:(hc + 1) * P].astype(np.float16)
    ident = np.eye(TLOC, dtype=f)
    sl = np.asarray(src_lengths)
    in_maps = []
    for c in range(NCORES):
        b, th = c // 2, c % 2
        t0 = th * TLOC
        maskc = (np.arange(S)[:, None] < int(sl[b])).astype(f)  # (S,1)
        maskc = np.ascontiguousarray(maskc.reshape(SB, P).T)    # (128,4)
        in_maps.append({
            "qT_l": np.ascontiguousarray(
                np.asarray(query[b, t0:t0 + TLOC, :], f).T),
            "encT_l": np.ascontiguousarray(
                np.asarray(encoder_outputs[b], f).T),
            "enc_l": np.ascontiguousarray(np.asarray(encoder_outputs[b], f)),
            "wsT": wsT,
            "whT": whT,
            "woutT": woutT,
            "v16": v16,
            "maskc": maskc,
            "ident": ident,
        })
    return in_maps


def kernel(query, encoder_outputs, src_lengths, Ws, Wh, v, Wout):
    from concourse.bass_utils import run_bass_kernel_spmd

    nc = build_module()
    in_maps = make_in_maps(query, encoder_outputs, src_lengths, Ws, Wh, v, Wout)
    res = run_bass_kernel_spmd(nc, in_maps, core_ids=list(range(NCORES))).results
    out = np.empty((B, T, H), np.float32)
    for c in range(NCORES):
        b, th = c // 2, c % 2
        t0 = th * TLOC
        out[b, t0:t0 + TLOC, :] = res[c]["out_l"]
    return out


# revision 19
# speedup vs baseline: 1.1127x; 1.1127x over previous
"""Bahdanau additive attention on 8 Trainium2 NeuronCores.

Problem shapes (hardcoded): B=4, T=128, S=512, H=256, fp32.

Sharding: data-parallel over (batch, T-half): core c handles b = c//2,
t in [64*(c%2), 64*(c%2)+64).  Every core runs the same SPMD program on
its own shard; weights are replicated.  No collectives.

Per-core algorithm (T_loc=64, S=512, H=256):
  peT[h,s] = (Wh @ enc^T)[h,s]        fp32 matmuls
  pqT[h,t] = (Ws @ q^T)[h,t]          fp32 matmuls
  For each t:  Y[h,s] = peT[h,s] + pqT[h,t]   (tensor_scalar_add with the
               pq column as the per-partition scalar operand, alternating
               between the DVE and Pool engines)
  X = tanh(Y)                          ACT on big (128 x 8192) tiles, fp16 out
  e[t,s] = sum_h v[h]*X[h,s]           PE fp16, shifted-vz stationary trick
  P = exp(e)                           no max-subtraction: |e| <= ||v||_1
  PT = P^T (PE transpose), PTm = PT * mask[s]  (per-partition mul)
  Z[t] = sum_s PTm[s,t]*mask[s]        PE matmul, out (64x1) t-on-partition
  c[t,:] = (1/Z[t]) * sum_s PTm[s,t]*enc[s,:]
  attn = tanh([q,c] @ Wout^T)          catT = [qT; cT], fp32 matmuls
"""

import numpy as np

B, T, S, H = 4, 128, 512, 256
TLOC = 64          # T rows per core
NCORES = 8
TGS = 16           # t's per tanh group
NG = TLOC // TGS   # 8 groups
P = 128            # partitions
HC = H // P        # 2 h-chunks
SB = S // P        # 4 s-blocks
FC = (2 * H) // P  # 4 f-chunks of cat=[q,c]

_CACHE = {}


def build_module():
    """Build + compile the SPMD Bass module (same program for all cores)."""
    if "nc" in _CACHE:
        return _CACHE["nc"]

    import concourse.bass as bass
    import concourse.tile as tile
    from concourse import bacc, mybir

    f32 = mybir.dt.float32
    f16 = mybir.dt.float16
    f32r = mybir.dt.float32r
    AF = mybir.ActivationFunctionType

    nc = bacc.Bacc(
        "TRN2",
        target_bir_lowering=False,
        debug=False,
        enable_asserts=False,
        num_devices=NCORES,
    )

    d_qT = nc.dram_tensor("qT_l", (H, TLOC), f32, kind="ExternalInput").ap()
    d_encT = nc.dram_tensor("encT_l", (H, S), f16, kind="ExternalInput").ap()
    d_enc = nc.dram_tensor("enc_l", (S, H), f32, kind="ExternalInput").ap()
    d_wsT = nc.dram_tensor("wsT", (H, H), f32, kind="ExternalInput").ap()
    d_whT = nc.dram_tensor("whT", (H, H), f16, kind="ExternalInput").ap()
    d_woutT = nc.dram_tensor("woutT", (2 * H, H), f32, kind="ExternalInput").ap()
    d_v24 = nc.dram_tensor("v24", (HC * P, 32), f16, kind="ExternalInput").ap()
    d_mask = nc.dram_tensor("maskc", (P, SB), f32, kind="ExternalInput").ap()
    d_ident = nc.dram_tensor("ident", (TLOC, TLOC), f32, kind="ExternalInput").ap()
    d_out = nc.dram_tensor("out_l", (TLOC, H), f32, kind="ExternalOutput").ap()

    with tile.TileContext(nc) as tc:
        from contextlib import ExitStack

        with ExitStack() as ctx:
            consts = ctx.enter_context(tc.tile_pool(name="consts", bufs=1))
            proj = ctx.enter_context(tc.tile_pool(name="proj", bufs=1))
            ypool = ctx.enter_context(tc.tile_pool(name="ypool", bufs=1))
            xpool = ctx.enter_context(tc.tile_pool(name="xpool", bufs=2))
            tail = ctx.enter_context(tc.tile_pool(name="tail", bufs=1))
            psA = ctx.enter_context(tc.tile_pool(name="psA", bufs=1, space="PSUM"))
            psE = ctx.enter_context(tc.tile_pool(name="psE", bufs=1, space="PSUM"))
            psE8 = ctx.enter_context(tc.tile_pool(name="psE8", bufs=3, space="PSUM"))
            psT = ctx.enter_context(tc.tile_pool(name="psT", bufs=3, space="PSUM"))

            # ---- load constants / inputs ----
            ws_sb = []
            wh_sb = []
            wout_sb = []
            qT_sb = []
            encT_sb = []
            enc_sb = []
            for kc in range(HC):
                t1 = consts.tile([P, H], f32, name=f"ws_sb{kc}")
                nc.sync.dma_start(t1[:], d_wsT[kc * P:(kc + 1) * P, :])
                ws_sb.append(t1)
                t2 = consts.tile([P, H], f16, name=f"wh_sb{kc}")
                nc.sync.dma_start(t2[:], d_whT[kc * P:(kc + 1) * P, :])
                wh_sb.append(t2)
                t3 = consts.tile([P, TLOC], f32, name=f"qT_sb{kc}")
                nc.sync.dma_start(t3[:], d_qT[kc * P:(kc + 1) * P, :])
                qT_sb.append(t3)
                t4 = consts.tile([P, S], f16, name=f"encT_sb{kc}")
                nc.sync.dma_start(t4[:], d_encT[kc * P:(kc + 1) * P, :])
                encT_sb.append(t4)
            for fc in range(FC):
                t5 = consts.tile([P, H], f32, name=f"wout_sb{fc}")
                nc.sync.dma_start(t5[:], d_woutT[fc * P:(fc + 1) * P, :])
                wout_sb.append(t5)
            for sb in range(SB):
                t6 = consts.tile([P, H], f32, name=f"enc_sb{sb}")
                nc.sync.dma_start(t6[:], d_enc[sb * P:(sb + 1) * P, :])
                enc_sb.append(t6)
            v24_sb = []
            for hc in range(HC):
                t6v = consts.tile([P, 32], f16, name=f"v24_sb{hc}")
                nc.sync.dma_start(t6v[:], d_v24[hc * P:(hc + 1) * P, :])
                v24_sb.append(t6v)
            mask_sb = consts.tile([P, SB], f32)
            nc.sync.dma_start(mask_sb[:], d_mask[:, :])
            ident_sb = consts.tile([TLOC, TLOC], f32)
            nc.sync.dma_start(ident_sb[:], d_ident[:, :])

            # ---- projections ----
            # peT[oc] (128 x 512): peT[o,s] = sum_h Wh[o,h] * encT[h,s]
            peT_sb = []
            for oc in range(HC):
                pe_ps = psA.tile([P, S], f32, name=f"pe_ps{oc}", tag="pe_ps")
                for kc in range(HC):
                    nc.tensor.matmul(
                        pe_ps[:],
                        lhsT=wh_sb[kc][:, oc * P:(oc + 1) * P],
                        rhs=encT_sb[kc][:],
                        start=(kc == 0),
                        stop=(kc == HC - 1),
                    )
                t7 = proj.tile([P, S], f16, name=f"peT_sb{oc}")
                nc.scalar.copy(t7[:], pe_ps[:])
                peT_sb.append(t7)

            # pqT[oc] (128 x 64): pqT[o,t] = sum_h Ws[o,h] * qT[h,t]  (fp32)
            pqT_sb = []
            for oc in range(HC):
                pq_ps = psT.tile([P, TLOC], f32, name=f"pq_ps{oc}", tag="tail")
                for kc in range(HC):
                    nc.tensor.matmul(
                        pq_ps[:],
                        lhsT=ws_sb[kc][:, oc * P:(oc + 1) * P],
                        rhs=qT_sb[kc][:],
                        start=(kc == 0),
                        stop=(kc == HC - 1),
                    )
                t8 = proj.tile([P, TLOC], f32, name=f"pqT_sb{oc}")
                nc.scalar.copy(t8[:], pq_ps[:])
                pqT_sb.append(t8)

            # ---- main loop: Y = pe + pq_t ; X = tanh(Y) ; e = v^T X ----
            # e rows come from M=8 matmuls with a shifted-column stationary
            # operand: v16[hc] is (128 x 16) with v[hc] at column 8, so
            # lhsT = v16[:, 8-jj:16-jj] has v in column jj -> the matmul
            # deposits row jj = v^T X_t (zeros elsewhere) of an (8 x 512)
            # PSUM tile, accumulating over hc.  8-row tiles are dense in
            # partitions, so a single DVE copy moves each to SBUF and PE
            # mini-transposes assemble eT (s-major) for the softmax tail.
            GS = [4, 8, 16, 16, 16, 4]   # staggered group sizes (sum 64)
            etiles = {}
            eT_ps = psE.tile([P, SB * TLOC], f32)  # (128 x 256) eT cols
            e8_sbs = []
            t0g = 0
            for g, tgs in enumerate(GS):
                xs = []
                for hc in range(HC):
                    y = ypool.tile([P, TGS * S], f16, name=f"y_{g}_{hc}",
                                   tag=f"y{hc}")
                    for j in range(tgs):
                        t = t0g + j
                        nc.vector.tensor_scalar_add(
                            y[:, j * S:(j + 1) * S],
                            peT_sb[hc][:],
                            pqT_sb[hc][:, t:t + 1],
                        )
                    x = xpool.tile([P, TGS * S], f16, name=f"x_{g}_{hc}",
                                   tag=f"x{hc}")
                    nc.scalar.activation(x[:, 0:tgs * S], y[:, 0:tgs * S],
                                         AF.Tanh)
                    xs.append(x)
                # subgroups of 16 rows; a group smaller than 16 contributes
                # a partial subgroup, completed by later groups
                for j in range(tgs):
                    t = t0g + j
                    u, jj = t // 16, t % 16
                    if jj == 0:
                        etiles[u] = psE8.tile([16, S], f32, name=f"e_{u}",
                                              tag="e_rows")
                    for hc in range(HC):
                        nc.tensor.matmul(
                            etiles[u][:, :],
                            lhsT=v24_sb[hc][:, 16 - jj:32 - jj],
                            rhs=xs[hc][:, j * S:(j + 1) * S],
                            start=(hc == 0 and jj == 0),
                            stop=(hc == HC - 1 and jj == 15),
                            skip_group_check=True,
                        )
                    if jj == 15:
                        e8 = tail.tile([16, S], f32, name=f"e8_{u}",
                                       tag="e8sb", bufs=2)
                        nc.vector.tensor_copy(e8[:], etiles[u][:])
                        for sb in range(SB):
                            nc.tensor.transpose(
                                eT_ps[:, sb * TLOC + u * 16:
                                      sb * TLOC + u * 16 + 16],
                                e8[:, sb * P:(sb + 1) * P],
                                ident_sb[0:16, 0:16],
                            )
                t0g += tgs

            # ---- softmax tail ----
            # eT_ps is (s-part x t-free); exp all chunks in one ACT op,
            # then per-partition masking per s-block chunk.
            pt_sb = tail.tile([P, SB * TLOC], f32)
            nc.scalar.activation(pt_sb[:], eT_ps[:], AF.Exp)
            ptm_sb = []
            for sb in range(SB):
                t9 = tail.tile([P, TLOC], f32, name=f"ptm_sb{sb}")
                nc.vector.tensor_scalar_mul(
                    t9[:],
                    pt_sb[:, sb * TLOC:(sb + 1) * TLOC],
                    mask_sb[:, sb:sb + 1],
                )
                ptm_sb.append(t9)

            z_ps = psT.tile([TLOC, 1], f32, tag="tail")
            for sb in range(SB):
                nc.tensor.matmul(
                    z_ps[:],
                    lhsT=ptm_sb[sb][:],
                    rhs=mask_sb[:, sb:sb + 1],
                    start=(sb == 0),
                    stop=(sb == SB - 1),
                )
            r_sb = tail.tile([TLOC, 1], f32)
            nc.vector.reciprocal(r_sb[:], z_ps[:])

            cun_ps = psT.tile([TLOC, H], f32, tag="tail")
            for sb in range(SB):
                nc.tensor.matmul(
                    cun_ps[:],
                    lhsT=ptm_sb[sb][:],
                    rhs=enc_sb[sb][:],
                    start=(sb == 0),
                    stop=(sb == SB - 1),
                )
            c_sb = tail.tile([TLOC, H], f32)
            nc.vector.tensor_scalar_mul(c_sb[:], cun_ps[:], r_sb[:])

            ct_ps = psT.tile([P, 2 * TLOC], f32, tag="tail")
            for i in range(HC):
                nc.tensor.transpose(
                    ct_ps[:, i * TLOC:(i + 1) * TLOC],
                    c_sb[:, i * P:(i + 1) * P],
                    ident_sb[:],
                )
            ct_sb = tail.tile([P, 2 * TLOC], f32)
            nc.vector.tensor_copy(ct_sb[:], ct_ps[:])

            attn_ps = psT.tile([TLOC, H], f32, tag="tail")
            cat_tiles = [
                qT_sb[0][:],
                qT_sb[1][:],
                ct_sb[:, 0:TLOC],
                ct_sb[:, TLOC:2 * TLOC],
            ]
            for fc in range(FC):
                nc.tensor.matmul(
                    attn_ps[:],
                    lhsT=cat_tiles[fc],
                    rhs=wout_sb[fc][:],
                    start=(fc == 0),
                    stop=(fc == FC - 1),
                )
            o_sb = tail.tile([TLOC, H], f32)
            nc.scalar.activation(o_sb[:], attn_ps[:], AF.Tanh)
            nc.sync.dma_start(d_out[:, :], o_sb[:])

    nc.compile()
    _CACHE["nc"] = nc
    return nc


def make_in_maps(query, encoder_outputs, src_lengths, Ws, Wh, v, Wout):
    """Host-side shard/layout prep: per-core input dict (all fp32, C-order)."""
    f = np.float32
    wsT = np.ascontiguousarray(np.asarray(Ws, f).T)
    whT = np.ascontiguousarray(np.asarray(Wh, f).T)
    woutT = np.ascontiguousarray(np.asarray(Wout, f).T)
    v24 = np.zeros((HC * P, 32), np.float16)
    for hc in range(HC):
        v24[hc * P:(hc + 1) * P, 16] = np.asarray(v, np.float32)[
            hc * P:(hc + 1) * P].astype(np.float16)
    ident = np.eye(TLOC, dtype=f)
    sl = np.asarray(src_lengths)
    in_maps = []
    for c in range(NCORES):
        b, th = c // 2, c % 2
        t0 = th * TLOC
        maskc = (np.arange(S)[:, None] < int(sl[b])).astype(f)  # (S,1)
        maskc = np.ascontiguousarray(maskc.reshape(SB, P).T)    # (128,4)
        in_maps.append({
            "qT_l": np.ascontiguousarray(
                np.asarray(query[b, t0:t0 + TLOC, :], f).T),
            "encT_l": np.ascontiguousarray(
                np.asarray(encoder_outputs[b], np.float16).T),
            "enc_l": np.ascontiguousarray(np.asarray(encoder_outputs[b], f)),
            "wsT": wsT,
            "whT": whT.astype(np.float16),
            "woutT": woutT,
            "v24": v24,
            "maskc": maskc,
            "ident": ident,
        })
    return in_maps


def kernel(query, encoder_outputs, src_lengths, Ws, Wh, v, Wout):
    from concourse.bass_utils import run_bass_kernel_spmd

    nc = build_module()
    in_maps = make_in_maps(query, encoder_outputs, src_lengths, Ws, Wh, v, Wout)
    res = run_bass_kernel_spmd(nc, in_maps, core_ids=list(range(NCORES))).results
    out = np.empty((B, T, H), np.float32)
    for c in range(NCORES):
        b, th = c // 2, c % 2
        t0 = th * TLOC
        out[b, t0:t0 + TLOC, :] = res[c]["out_l"]
    return out


# revision 23
# speedup vs baseline: 1.1983x; 1.0769x over previous
"""Bahdanau additive attention on 8 Trainium2 NeuronCores.

Problem shapes (hardcoded): B=4, T=128, S=512, H=256, fp32.

Sharding: data-parallel over (batch, T-half): core c handles b = c//2,
t in [64*(c%2), 64*(c%2)+64).  Every core runs the same SPMD program on
its own shard; weights are replicated.  No collectives.

Per-core algorithm (T_loc=64, S=512, H=256):
  peT[h,s] = (Wh @ enc^T)[h,s]        fp32 matmuls
  pqT[h,t] = (Ws @ q^T)[h,t]          fp32 matmuls
  For each t:  Y[h,s] = peT[h,s] + pqT[h,t]   (tensor_scalar_add with the
               pq column as the per-partition scalar operand, alternating
               between the DVE and Pool engines)
  X = tanh(Y)                          ACT on big (128 x 8192) tiles, fp16 out
  e[t,s] = sum_h v[h]*X[h,s]           PE fp16, shifted-vz stationary trick
  P = exp(e)                           no max-subtraction: |e| <= ||v||_1
  PT = P^T (PE transpose), PTm = PT * mask[s]  (per-partition mul)
  Z[t] = sum_s PTm[s,t]*mask[s]        PE matmul, out (64x1) t-on-partition
  c[t,:] = (1/Z[t]) * sum_s PTm[s,t]*enc[s,:]
  attn = tanh([q,c] @ Wout^T)          catT = [qT; cT], fp32 matmuls
"""

import numpy as np

B, T, S, H = 4, 128, 512, 256
TLOC = 64          # T rows per core
NCORES = 8
TGS = 16           # t's per tanh group
NG = TLOC // TGS   # 8 groups
P = 128            # partitions
HC = H // P        # 2 h-chunks
SB = S // P        # 4 s-blocks
FC = (2 * H) // P  # 4 f-chunks of cat=[q,c]

_CACHE = {}


def build_module():
    """Build + compile the SPMD Bass module (same program for all cores)."""
    if "nc" in _CACHE:
        return _CACHE["nc"]

    import concourse.bass as bass
    import concourse.tile as tile
    from concourse import bacc, mybir

    f32 = mybir.dt.float32
    f16 = mybir.dt.float16
    f32r = mybir.dt.float32r
    AF = mybir.ActivationFunctionType

    nc = bacc.Bacc(
        "TRN2",
        target_bir_lowering=False,
        debug=False,
        enable_asserts=False,
        num_devices=NCORES,
    )

    d_qT = nc.dram_tensor("qT_l", (H, TLOC), f16, kind="ExternalInput").ap()
    d_encT = nc.dram_tensor("encT_l", (H, S), f16, kind="ExternalInput").ap()
    d_enc = nc.dram_tensor("enc_l", (S, H), f16, kind="ExternalInput").ap()
    d_wsT = nc.dram_tensor("wsT", (H, H), f16, kind="ExternalInput").ap()
    d_whT = nc.dram_tensor("whT", (H, H), f16, kind="ExternalInput").ap()
    d_woutT = nc.dram_tensor("woutT", (2 * H, H), f16, kind="ExternalInput").ap()
    d_v24 = nc.dram_tensor("v24", (HC * P, 32), f16, kind="ExternalInput").ap()
    d_mask = nc.dram_tensor("maskc", (P, SB), f16, kind="ExternalInput").ap()
    d_ident = nc.dram_tensor("ident", (TLOC, TLOC), f16, kind="ExternalInput").ap()
    d_out = nc.dram_tensor("out_l", (TLOC, H), f32, kind="ExternalOutput").ap()

    with tile.TileContext(nc) as tc:
        from contextlib import ExitStack

        with ExitStack() as ctx:
            consts = ctx.enter_context(tc.tile_pool(name="consts", bufs=1))
            proj = ctx.enter_context(tc.tile_pool(name="proj", bufs=1))
            ypool = ctx.enter_context(tc.tile_pool(name="ypool", bufs=1))
            xpool = ctx.enter_context(tc.tile_pool(name="xpool", bufs=2))
            tail = ctx.enter_context(tc.tile_pool(name="tail", bufs=1))
            psA = ctx.enter_context(tc.tile_pool(name="psA", bufs=1, space="PSUM"))
            psE = ctx.enter_context(tc.tile_pool(name="psE", bufs=1, space="PSUM"))
            psE8 = ctx.enter_context(tc.tile_pool(name="psE8", bufs=3, space="PSUM"))
            psT = ctx.enter_context(tc.tile_pool(name="psT", bufs=3, space="PSUM"))

            # ---- load constants / inputs (critical-path order: encT/whT
            # feed the pe projection that gates the whole main loop) ----
            encT_sb = []
            wh_sb = []
            for kc in range(HC):
                t4 = consts.tile([P, S], f16, name=f"encT_sb{kc}")
                nc.sync.dma_start(t4[:], d_encT[kc * P:(kc + 1) * P, :])
                encT_sb.append(t4)
            for kc in range(HC):
                t2 = consts.tile([P, H], f16, name=f"wh_sb{kc}")
                nc.sync.dma_start(t2[:], d_whT[kc * P:(kc + 1) * P, :])
                wh_sb.append(t2)
            qT_sb = []
            ws_sb = []
            for kc in range(HC):
                t3 = consts.tile([P, TLOC], f16, name=f"qT_sb{kc}")
                nc.sync.dma_start(t3[:], d_qT[kc * P:(kc + 1) * P, :])
                qT_sb.append(t3)
            for kc in range(HC):
                t1 = consts.tile([P, H], f16, name=f"ws_sb{kc}")
                nc.sync.dma_start(t1[:], d_wsT[kc * P:(kc + 1) * P, :])
                ws_sb.append(t1)
            v24_sb = []
            for hc in range(HC):
                t6v = consts.tile([P, 32], f16, name=f"v24_sb{hc}")
                nc.sync.dma_start(t6v[:], d_v24[hc * P:(hc + 1) * P, :])
                v24_sb.append(t6v)
            ident_sb = consts.tile([TLOC, TLOC], f16)
            nc.sync.dma_start(ident_sb[:], d_ident[:, :])
            mask_sb = consts.tile([P, SB], f16)
            nc.sync.dma_start(mask_sb[:], d_mask[:, :])
            maskf_sb = consts.tile([P, SB], f32)
            nc.vector.tensor_copy(maskf_sb[:], mask_sb[:])
            enc_sb = []
            for sb in range(SB):
                t6 = consts.tile([P, H], f16, name=f"enc_sb{sb}")
                nc.sync.dma_start(t6[:], d_enc[sb * P:(sb + 1) * P, :])
                enc_sb.append(t6)
            wout_sb = []
            for fc in range(FC):
                t5 = consts.tile([P, H], f16, name=f"wout_sb{fc}")
                nc.sync.dma_start(t5[:], d_woutT[fc * P:(fc + 1) * P, :])
                wout_sb.append(t5)

            # ---- projections ----
            # peT[oc] (128 x 512): peT[o,s] = sum_h Wh[o,h] * encT[h,s]
            peT_sb = []
            for oc in range(HC):
                pe_ps = psA.tile([P, S], f32, name=f"pe_ps{oc}", tag="pe_ps")
                for kc in range(HC):
                    nc.tensor.matmul(
                        pe_ps[:],
                        lhsT=wh_sb[kc][:, oc * P:(oc + 1) * P],
                        rhs=encT_sb[kc][:],
                        start=(kc == 0),
                        stop=(kc == HC - 1),
                    )
                t7 = proj.tile([P, S], f16, name=f"peT_sb{oc}")
                nc.vector.tensor_copy(t7[:], pe_ps[:])
                peT_sb.append(t7)

            # pqT[oc] (128 x 64): pqT[o,t] = sum_h Ws[o,h] * qT[h,t]  (fp32)
            pqT_sb = []
            for oc in range(HC):
                pq_ps = psT.tile([P, TLOC], f32, name=f"pq_ps{oc}", tag="tail")
                for kc in range(HC):
                    nc.tensor.matmul(
                        pq_ps[:],
                        lhsT=ws_sb[kc][:, oc * P:(oc + 1) * P],
                        rhs=qT_sb[kc][:],
                        start=(kc == 0),
                        stop=(kc == HC - 1),
                    )
                t8 = proj.tile([P, TLOC], f32, name=f"pqT_sb{oc}")
                nc.vector.tensor_copy(t8[:], pq_ps[:])
                pqT_sb.append(t8)

            # ---- main loop: Y = pe + pq_t ; X = tanh(Y) ; e = v^T X ----
            # e rows come from M=8 matmuls with a shifted-column stationary
            # operand: v16[hc] is (128 x 16) with v[hc] at column 8, so
            # lhsT = v16[:, 8-jj:16-jj] has v in column jj -> the matmul
            # deposits row jj = v^T X_t (zeros elsewhere) of an (8 x 512)
            # PSUM tile, accumulating over hc.  8-row tiles are dense in
            # partitions, so a single DVE copy moves each to SBUF and PE
            # mini-transposes assemble eT (s-major) for the softmax tail.
            GS = [4, 8, 16, 16, 16, 4]   # staggered group sizes (sum 64)
            etiles = {}
            eT_ps = psE.tile([P, SB * TLOC], f16)  # (128 x 256) eT cols
            e8_sbs = []
            t0g = 0
            for g, tgs in enumerate(GS):
                xs = []
                for hc in range(HC):
                    y = ypool.tile([P, TGS * S], f16, name=f"y_{g}_{hc}",
                                   tag=f"y{hc}")
                    for j in range(tgs):
                        t = t0g + j
                        nc.vector.tensor_scalar_add(
                            y[:, j * S:(j + 1) * S],
                            peT_sb[hc][:],
                            pqT_sb[hc][:, t:t + 1],
                        )
                    x = xpool.tile([P, TGS * S], f16, name=f"x_{g}_{hc}",
                                   tag=f"x{hc}")
                    nc.scalar.activation(x[:, 0:tgs * S], y[:, 0:tgs * S],
                                         AF.Tanh)
                    xs.append(x)
                # subgroups of 16 rows; a group smaller than 16 contributes
                # a partial subgroup, completed by later groups
                for j in range(tgs):
                    t = t0g + j
                    u, jj = t // 16, t % 16
                    if jj == 0:
                        etiles[u] = psE8.tile([16, S], f32, name=f"e_{u}",
                                              tag="e_rows")
                    for hc in range(HC):
                        nc.tensor.matmul(
                            etiles[u][:, :],
                            lhsT=v24_sb[hc][:, 16 - jj:32 - jj],
                            rhs=xs[hc][:, j * S:(j + 1) * S],
                            start=(hc == 0 and jj == 0),
                            stop=(hc == HC - 1 and jj == 15),
                            skip_group_check=True,
                        )
                    if jj == 15:
                        e8 = tail.tile([16, S], f16, name=f"e8_{u}",
                                       tag="e8sb", bufs=2)
                        nc.vector.tensor_copy(e8[:], etiles[u][:])
                        for sb in range(SB):
                            nc.tensor.transpose(
                                eT_ps[:, sb * TLOC + u * 16:
                                      sb * TLOC + u * 16 + 16],
                                e8[:, sb * P:(sb + 1) * P],
                                ident_sb[0:16, 0:16],
                            )
                t0g += tgs

            # ---- softmax tail ----
            # eT_ps is (s-part x t-free); exp all chunks in one ACT op,
            # then per-partition masking per s-block chunk.
            # exp(e - 4) in fp16: |e| <= ||v||_1 ~ 12.8 so exp(e-4) < 7e3
            # stays in fp16 range; the e^-4 factor cancels in alpha = P/Z.
            negc_sb = consts.tile([P, 1], f32)
            nc.vector.memset(negc_sb[:], -4.0)
            pt_sb = tail.tile([P, SB * TLOC], f16)
            nc.scalar.activation(pt_sb[:], eT_ps[:], AF.Exp, bias=negc_sb[:, 0:1])
            ptm_sb = []
            for sb in range(SB):
                t9 = tail.tile([P, TLOC], f16, name=f"ptm_sb{sb}")
                nc.vector.tensor_scalar_mul(
                    t9[:],
                    pt_sb[:, sb * TLOC:(sb + 1) * TLOC],
                    maskf_sb[:, sb:sb + 1],
                )
                ptm_sb.append(t9)

            z_ps = psT.tile([TLOC, 1], f32, tag="tail")
            for sb in range(SB):
                nc.tensor.matmul(
                    z_ps[:],
                    lhsT=ptm_sb[sb][:],
                    rhs=mask_sb[:, sb:sb + 1],
                    start=(sb == 0),
                    stop=(sb == SB - 1),
                )
            r_sb = tail.tile([TLOC, 1], f32)
            nc.vector.reciprocal(r_sb[:], z_ps[:])

            cun_ps = psT.tile([TLOC, H], f32, tag="tail")
            for sb in range(SB):
                nc.tensor.matmul(
                    cun_ps[:],
                    lhsT=ptm_sb[sb][:],
                    rhs=enc_sb[sb][:],
                    start=(sb == 0),
                    stop=(sb == SB - 1),
                )
            c_sb = tail.tile([TLOC, H], f16)
            nc.vector.tensor_scalar_mul(c_sb[:], cun_ps[:], r_sb[:])

            ct_ps = psT.tile([P, 2 * TLOC], f16, tag="tail")
            for i in range(HC):
                nc.tensor.transpose(
                    ct_ps[:, i * TLOC:(i + 1) * TLOC],
                    c_sb[:, i * P:(i + 1) * P],
                    ident_sb[:],
                )
            ct_sb = tail.tile([P, 2 * TLOC], f16)
            nc.vector.tensor_copy(ct_sb[:], ct_ps[:])

            attn_ps = psT.tile([TLOC, H], f32, tag="tail")
            cat_tiles = [
                qT_sb[0][:],
                qT_sb[1][:],
                ct_sb[:, 0:TLOC],
                ct_sb[:, TLOC:2 * TLOC],
            ]
            for fc in range(FC):
                nc.tensor.matmul(
                    attn_ps[:],
                    lhsT=cat_tiles[fc],
                    rhs=wout_sb[fc][:],
                    start=(fc == 0),
                    stop=(fc == FC - 1),
                )
            o_sb = tail.tile([TLOC, H], f32)
            nc.scalar.activation(o_sb[:], attn_ps[:], AF.Tanh)
            nc.sync.dma_start(d_out[:, :], o_sb[:])

    nc.compile()
    _CACHE["nc"] = nc
    return nc


def make_in_maps(query, encoder_outputs, src_lengths, Ws, Wh, v, Wout):
    """Host-side shard/layout prep: per-core input dict (all fp32, C-order)."""
    f = np.float32
    wsT = np.ascontiguousarray(np.asarray(Ws, np.float16).T)
    whT = np.ascontiguousarray(np.asarray(Wh, f).T)
    woutT = np.ascontiguousarray(np.asarray(Wout, np.float16).T)
    v24 = np.zeros((HC * P, 32), np.float16)
    for hc in range(HC):
        v24[hc * P:(hc + 1) * P, 16] = np.asarray(v, np.float32)[
            hc * P:(hc + 1) * P].astype(np.float16)
    ident = np.eye(TLOC, dtype=np.float16)
    sl = np.asarray(src_lengths)
    in_maps = []
    for c in range(NCORES):
        b, th = c // 2, c % 2
        t0 = th * TLOC
        maskc = (np.arange(S)[:, None] < int(sl[b])).astype(np.float16)
        maskc = np.ascontiguousarray(maskc.reshape(SB, P).T)    # (128,4)
        in_maps.append({
            "qT_l": np.ascontiguousarray(np.asarray(
                query[b, t0:t0 + TLOC, :], np.float16).T),
            "encT_l": np.ascontiguousarray(
                np.asarray(encoder_outputs[b], np.float16).T),
            "enc_l": np.ascontiguousarray(
                np.asarray(encoder_outputs[b], np.float16)),
            "wsT": wsT,
            "whT": whT.astype(np.float16),
            "woutT": woutT,
            "v24": v24,
            "maskc": maskc,
            "ident": ident,
        })
    return in_maps


def kernel(query, encoder_outputs, src_lengths, Ws, Wh, v, Wout):
    from concourse.bass_utils import run_bass_kernel_spmd

    nc = build_module()
    in_maps = make_in_maps(query, encoder_outputs, src_lengths, Ws, Wh, v, Wout)
    res = run_bass_kernel_spmd(nc, in_maps, core_ids=list(range(NCORES))).results
    out = np.empty((B, T, H), np.float32)
    for c in range(NCORES):
        b, th = c // 2, c % 2
        t0 = th * TLOC
        out[b, t0:t0 + TLOC, :] = res[c]["out_l"]
    return out
